# revision 32
# baseline (speedup 1.0000x reference)
"""LocalMean 5x5 box filter (reflect pad) on TRN2, data-parallel over 8 cores.

Full input:  image (32, 3, 512, 512) fp32
Full output: same shape, 5x5 mean with reflect padding on H and W.

Sharding: batch dim 32 -> 4 images per core (12 channel planes of 512x512).

v6 design (harness gate is rel_err < 2e-2, so no exact hi/lo split needed):
  - Host converts input fp32->fp16 (and output fp16->fp32), reflect-pads
    the columns to 516 and zero-pads 10 extra rows. Device I/O is all
    fp16: halves HBM traffic, no on-device dtype conversion, no on-device
    padding. Total numeric error ~5*2^-12 relative to each window mean.
  - Uniform vertical chunking: 5 chunks of 110 rows at stride 103 (the
    zero rows keep chunk 4 in bounds), so ONE 3-level load descriptor
    per plane. Loads go via SWDGE (gpsimd dma_start): HWDGE HBM-read
    packets are confined to SDMA engines 64-67, while SWDGE spreads
    across all 16.
  - Horizontal 5-tap: DVE computes Q[f]=Xp[f]+Xp[f+2] and E[w]=Q[w]+
    Xp[w+4] (both 2x-packed 16-bit). The remaining Q[w+1] term is a
    second accumulating matmul per group (the PE streams the rhs at an
    odd element offset for free, same band weights).
  - Vertical 5-tap via PE band matmuls, 5 uniform groups of K=110,
    M=(105,103,103,103,98), weights {0,1,2} exact in fp16.
  - PSUM->SBUF evac on ScalarE with the 1/25 scale, fp16 out, 2KB-bank
    pairs per ACTIVATE.
  - Stores: 3-level middle descriptor on SWDGE (multi-level HWDGE
    descriptors pin to engines 64-67), 2-level first/last on qSync HWDGE
    (those spread over engines 68-79).
"""

import numpy as np

import concourse.bass as bass
import concourse.mybir as mybir
import concourse.tile as tile
from concourse.tile import add_dep_helper
from concourse.bass_utils import run_bass_kernel_spmd

try:
    from bass_rust import AP as RustAP
except ImportError:  # pragma: no cover
    RustAP = None

F32 = mybir.dt.float32
F16 = mybir.dt.float16

N_CORES = 8
NB = 32
NBPC = NB // N_CORES
NCH = NBPC * 3
H = W = 512
PATCH = 5
PAD = 2
INV_AREA = 1.0 / float(PATCH * PATCH)

XTW = W + 2 * PAD  # 516 padded width
QW = W + 2  # 514 gap-pair width

# Row groups: (in_base, K, out_base, M). Chunks 0-3 are 128 rows at
# stride 124; the tail chunk is rows 496..511 (K=16).
GROUPS = [
    (0, 128, 0, 126),
    (124, 128, 126, 124),
    (248, 128, 250, 124),
    (372, 128, 374, 124),
    (496, 16, 498, 14),
]
_VM_IDX = [0, 1, 1, 1, 2]
MW = 126  # max group M (weight tile column count)


def _reflect(t, n):
    if t < 0:
        t = -t
    if t > n - 1:
        t = 2 * (n - 1) - t
    return t


def _v_matrix(in_base, k_rows, out_base, m_rows):
    v = np.zeros((128, MW), np.float32)
    for m in range(m_rows):
        r = out_base + m
        for t in range(r - PAD, r + PAD + 1):
            k = _reflect(t, H) - in_base
            assert 0 <= k < k_rows, (r, t, k)
            v[k, m] += 1.0
    return v


def _build_vmats():
    v = np.stack(
        [
            _v_matrix(*GROUPS[0]),
            _v_matrix(*GROUPS[1]),
            _v_matrix(*GROUPS[4]),
        ]
    )
    assert np.all(np.isin(v, [0.0, 1.0, 2.0]))
    return v.astype(np.float16)


VMATS16 = _build_vmats()


def _mk_ap(like_ap, offset, pattern):
    return RustAP(tensor=like_ap.tensor, offset=offset, ap=pattern)


def build_module(split_waits=True):
    nc = bass.Bass()
    # Input arrives host-padded: columns reflect-padded to 516.
    img = nc.dram_tensor("image", [NCH, H, XTW], F16, kind="ExternalInput")
    vm16 = nc.dram_tensor("vmats16", [3, 128, MW], F16, kind="ExternalInput")
    out = nc.dram_tensor("out", [NCH, H, W], F16, kind="ExternalOutput")

    with tile.TileContext(nc) as tc:
        with (
            tc.tile_pool(name="const", bufs=1) as constp,
            tc.tile_pool(name="xin", bufs=4) as xinp,
            tc.tile_pool(name="qp", bufs=4) as qpool,
            tc.tile_pool(name="ep", bufs=4) as epool,
            tc.tile_pool(name="outp", bufs=4) as outp,
            tc.tile_pool(name="psA", bufs=3, space=bass.MemorySpace.PSUM) as psA,
            tc.tile_pool(name="psT", bufs=2, space=bass.MemorySpace.PSUM) as psT,
        ):
            # Staging tile for the 14-row output tails of all planes: one
            # 3-level store at the end replaces 12 tiny per-plane stores.
            tstage = constp.tile([14, NCH * W], F16)
            tsv = tstage[:].rearrange("p (c f) -> p c f", c=NCH)

            # All planes' 16-row input tails in one upfront load (the
            # input is read-only): replaces 12 tiny per-plane descriptors.
            xtail = constp.tile([16, NCH * XTW], F16)
            xtv = xtail[:].rearrange("p (c f) -> p c f", c=NCH)
            nc.sync.dma_start(
                xtv,
                _mk_ap(
                    img[:],
                    (H - 16) * XTW,
                    [[XTW, 16], [H * XTW, NCH], [1, XTW]],
                ),
            )

            vt = constp.tile([128, 3 * MW], F16)
            vtr = vt[:].rearrange("p (i m) -> p i m", i=3)
            nc.sync.dma_start(
                vtr, _mk_ap(vm16[:], 0, [[MW, 128], [128 * MW, 3], [1, MW]])
            )

            # Warmup matmul consumes the weight tile right after its DMA.
            wup = psT.tile([128, 512], F32, tag="pt")
            warm = nc.tensor.matmul(
                wup[0:1, 0 : 3 * MW],
                vt[0:128, 0:1],
                vt[:],
                start=True,
                stop=True,
            )
            prev_mm = warm
            prev_dve = None
            prev_act = None
            prev_gps = None

            def chain(inst, which):
                nonlocal prev_dve, prev_act, prev_gps
                prevs = {"dve": prev_dve, "act": prev_act, "gps": prev_gps}
                p = prevs[which]
                if p is not None:
                    add_dep_helper(inst.ins, p.ins, sync=False, reason=which)
                if which == "dve":
                    prev_dve = inst
                elif which == "act":
                    prev_act = inst
                else:
                    prev_gps = inst
                return inst

            def mm_chain(inst):
                nonlocal prev_mm
                add_dep_helper(inst.ins, prev_mm.ins, sync=False, reason="pe order")
                prev_mm = inst
                return inst

            # DMA queues are FIFO per issuing engine: a store descriptor
            # waiting on an evac would block the NEXT plane's load sitting
            # behind it in the same queue. Issue loads PREFETCH planes
            # ahead of the compute so loads never queue behind stores.
            PREFETCH = 3
            xtiles = {}

            def issue_loads(c):
                x = xinp.tile([128, 4 * XTW], F16)
                xv = x[:].rearrange("p (a f) -> p a f", a=4)
                xtiles[c] = xv
                # Main load (chunks 0-3, 128 rows at stride 124) via
                # SWDGE: HWDGE HBM-read packets are confined to SDMA
                # engines 64-67, while SWDGE spreads across all 16.
                nc.gpsimd.dma_start(
                    xv[:, :, :],
                    _mk_ap(
                        img[:],
                        c * H * XTW,
                        [[XTW, 128], [124 * XTW, 4], [1, XTW]],
                    ),
                )

            for c in range(min(PREFETCH, NCH)):
                issue_loads(c)

            for c in range(NCH):
                if c + PREFETCH < NCH:
                    issue_loads(c + PREFETCH)
                xv = xtiles.pop(c)

                # Q[f] = Xp[f] + Xp[f+2]  (2x packed). Main chunks 0-3 at
                # full partition width; the 16-partition tail chunk runs
                # as its own small op (no memset of garbage partitions
                # needed that way).
                q = qpool.tile([128, 5 * QW], F16)
                qv = q[:].rearrange("p (a f) -> p a f", a=5)
                chain(
                    nc.vector.tensor_tensor(
                        qv[:, 0:4, 0:QW],
                        xv[:, 0:4, 0:QW],
                        xv[:, 0:4, 2 : 2 + QW],
                        mybir.AluOpType.add,
                    ),
                    "dve",
                )
                chain(
                    nc.vector.tensor_tensor(
                        qv[0:16, 4, 0:QW],
                        xtv[0:16, c, 0:QW],
                        xtv[0:16, c, 2 : 2 + QW],
                        mybir.AluOpType.add,
                    ),
                    "dve",
                )

                # E[w] = Q[w] + Xp[w+4]  (2x packed). The remaining Q[w+1]
                # term is a second accumulating matmul per group (the PE
                # streams the rhs at an odd element offset for free).
                e = epool.tile([128, 5 * W], F16)
                ev = e[:].rearrange("p (a f) -> p a f", a=5)
                chain(
                    nc.vector.tensor_tensor(
                        ev[:, 0:4, 0:W],
                        qv[:, 0:4, 0:W],
                        xv[:, 0:4, 4 : 4 + W],
                        mybir.AluOpType.add,
                    ),
                    "dve",
                )
                chain(
                    nc.vector.tensor_tensor(
                        ev[0:16, 4, 0:W],
                        qv[0:16, 4, 0:W],
                        xtv[0:16, c, 4 : 4 + W],
                        mybir.AluOpType.add,
                    ),
                    "dve",
                )

                # Vertical band matmuls, 2 accumulating per group:
                # PSUM_g = V_g^T @ E_g + V_g^T @ Q_g[w+1]
                pa1 = psA.tile([128, 1024], F32, tag="pa")
                pa2 = psA.tile([128, 1024], F32, tag="pa")
                pt = psT.tile([128, 512], F32, tag="pt")
                for g, (in_base, krows, out_base, m_rows) in enumerate(GROUPS):
                    # Write the full partition range the paired evac reads
                    # (extra weight columns are zero, so extra rows are 0).
                    mw = 126 if g < 4 else 14
                    kk = 128 if g < 4 else 16
                    if g < 4:
                        dst = (pa1 if g < 2 else pa2)[
                            0:mw, (g % 2) * W : (g % 2 + 1) * W
                        ]
                    else:
                        dst = pt[0:mw, :]
                    lhs = vtr[0:kk, _VM_IDX[g], 0:mw]
                    mm_chain(
                        nc.tensor.matmul(
                            dst,
                            lhs,
                            ev[0:kk, g, 0:W],
                            start=True,
                            stop=False,
                        )
                    )
                    mm_chain(
                        nc.tensor.matmul(
                            dst,
                            lhs,
                            qv[0:kk, g, 1 : 1 + W],
                            start=False,
                            stop=True,
                        )
                    )

                # Evacuate PSUM -> SBUF fp16 with the 1/25 scale on ScalarE.
                ot = outp.tile([128, 5 * W], F16)
                chain(nc.scalar.mul(ot[0:126, 0:1024], pa1[0:126, :], INV_AREA), "act")
                chain(
                    nc.scalar.mul(ot[0:126, 1024:2048], pa2[0:126, :], INV_AREA),
                    "act",
                )
                chain(
                    nc.scalar.mul(tsv[0:14, c, :], pt[0:14, :], INV_AREA), "act"
                )

                # Stores: g0 is 2-level on qSync HWDGE (engines 68-79);
                # the 3-level g1-g3 descriptor goes via SWDGE (multi-level
                # HWDGE descriptors pin to engines 64-67) except for the
                # last planes, where Sync is idle and SWDGE would extend
                # the end-of-kernel drain.
                ov = ot[:].rearrange("p (g f) -> p g f", g=5)
                nc.sync.dma_start(out[c, 0:126, :], ov[0:126, 0, :])
                midq = nc.gpsimd if c < NCH - 2 else nc.sync
                midq.dma_start(
                    _mk_ap(
                        out[:],
                        c * H * W + 126 * W,
                        [[W, 124], [124 * W, 3], [1, W]],
                    ),
                    ov[0:124, 1:4, :],
                )

            # Single 3-level store for all planes' 14-row tails.
            nc.sync.dma_start(
                _mk_ap(
                    out[:],
                    (H - 14) * W,
                    [[W, 14], [H * W, NCH], [1, W]],
                ),
                tsv[0:14, :, :],
            )

    if split_waits:
        _split_waits(nc)
    return nc


def _split_waits(nc):
    """Walrus legalization: each 64B ISA instruction has ONE sync-wait slot.

    Tile emits instructions with multiple semaphore waits; split the extras
    into standalone InstEventSemaphore sequencer waits (same engine queue,
    immediately before the instruction) which is semantically identical.
    """
    for fn in nc.m.functions:
        for b in fn.blocks:
            insts = b.instructions
            if not any(
                ins.sync_info and len(ins.sync_info.on_wait) > 1 for ins in insts
            ):
                continue
            new = []
            for ins in insts:
                si = ins.sync_info
                if si and len(si.on_wait) > 1:
                    waits = list(si.on_wait)
                    for w in waits[:-1]:
                        ev = mybir.InstEventSemaphore(
                            name=nc.get_next_instruction_name(),
                            engine=ins.engine,
                            ins=[],
                            outs=[],
                        )
                        ev.sync_info = mybir.SyncInfo(on_wait=[w], on_update=[])
                        new.append(ev)
                    si.on_wait = [waits[-1]]
                new.append(ins)
            b.instructions = new


_NC_CACHE = None


def _get_module():
    global _NC_CACHE
    if _NC_CACHE is None:
        _NC_CACHE = build_module()
    return _NC_CACHE


def kernel(image, _trace=False, _trace_kwargs=None):
    image = np.asarray(image)
    assert image.shape == (NB, 3, H, W), image.shape
    in_dtype = image.dtype
    img16 = np.pad(
        image.astype(np.float16),
        ((0, 0), (0, 0), (0, 0), (PAD, PAD)),
        mode="reflect",
    )

    nc = _get_module()
    in_maps = [
        {
            "image": img16[i * NBPC : (i + 1) * NBPC].reshape(NCH, H, XTW),
            "vmats16": VMATS16,
        }
        for i in range(N_CORES)
    ]
    res = run_bass_kernel_spmd(
        nc,
        in_maps,
        list(range(N_CORES)),
        trace=_trace,
        **(_trace_kwargs or {}),
    )
    full = np.concatenate(
        [res.results[i]["out"].reshape(NBPC, 3, H, W) for i in range(N_CORES)],
        axis=0,
    )
    out = full.astype(in_dtype, copy=False)
    if _trace:
        return out, res
    return out


# revision 33
# speedup vs baseline: 1.0165x; 1.0165x over previous
"""LocalMean 5x5 box filter (reflect pad) on TRN2, data-parallel over 8 cores.

Full input:  image (32, 3, 512, 512) fp32
Full output: same shape, 5x5 mean with reflect padding on H and W.

Sharding: batch dim 32 -> 4 images per core (12 channel planes of 512x512).

Design (the harness gate is rel_err < 2e-2, so no exact hi/lo split is
needed; measured rel_err ~9e-4):
  - Host converts input fp32->fp16 and reflect-pads the columns to 516;
    output is fp16 on device, upconverted on the host. Device HBM
    traffic is all fp16 (halves bytes, no on-device dtype conversion or
    padding).
  - Horizontal 5-tap: DVE computes Q[f]=Xp[f]+Xp[f+2] and E[w]=Q[w]+
    Xp[w+4], both in the 2x-packed 16-bit mode. The remaining Q[w+1]
    term is a second accumulating matmul per group (the PE streams the
    rhs at an odd element offset for free, same band weights).
  - Vertical 5-tap via PE band matmuls: groups of rows
    (126,124,124,124,14), K=(128x4,16), weights {0,1,2} exact in fp16.
  - PSUM->SBUF evacuation on ScalarE applies the 1/25 scale and the
    fp16 downconvert in one ACTIVATE per 2-bank PSUM pair.
  - DMA engine mapping (measured): HWDGE HBM-read packets are confined
    to SDMA engines 64-67 and multi-level HWDGE descriptors also pin to
    64-67, while SWDGE (gpsimd dma_start) spreads across all 16 engines
    and 2-level HWDGE writes spread over 68-79. Hence: main loads and
    the 3-level middle store go via SWDGE; g0 stores via qSync; all
    input tails are fetched in one upfront descriptor and all output
    tails staged in SBUF and stored in one final descriptor.
  - DMA queues are FIFO per issuing engine, so loads are issued 3
    planes ahead of compute to never sit behind a store descriptor
    that is still waiting on its evacuation.
"""

import numpy as np

import concourse.bass as bass
import concourse.mybir as mybir
import concourse.tile as tile
from concourse.tile import add_dep_helper
from concourse.bass_utils import run_bass_kernel_spmd

try:
    from bass_rust import AP as RustAP
except ImportError:  # pragma: no cover
    RustAP = None

F32 = mybir.dt.float32
F16 = mybir.dt.float16

N_CORES = 8
NB = 32
NBPC = NB // N_CORES
NCH = NBPC * 3
H = W = 512
PATCH = 5
PAD = 2
INV_AREA = 1.0 / float(PATCH * PATCH)

XTW = W + 2 * PAD  # 516 padded width
QW = W + 2  # 514 gap-pair width

# Row groups: (in_base, K, out_base, M). Chunks 0-3 are 128 rows at
# stride 124; the tail chunk is rows 496..511 (K=16).
GROUPS = [
    (0, 128, 0, 126),
    (124, 128, 126, 124),
    (248, 128, 250, 124),
    (372, 128, 374, 124),
    (496, 16, 498, 14),
]
_VM_IDX = [0, 1, 1, 1, 2]
MW = 126  # max group M (weight tile column count)


def _reflect(t, n):
    if t < 0:
        t = -t
    if t > n - 1:
        t = 2 * (n - 1) - t
    return t


def _v_matrix(in_base, k_rows, out_base, m_rows):
    v = np.zeros((128, MW), np.float32)
    for m in range(m_rows):
        r = out_base + m
        for t in range(r - PAD, r + PAD + 1):
            k = _reflect(t, H) - in_base
            assert 0 <= k < k_rows, (r, t, k)
            v[k, m] += 1.0
    return v


def _build_vmats():
    v = np.stack(
        [
            _v_matrix(*GROUPS[0]),
            _v_matrix(*GROUPS[1]),
            _v_matrix(*GROUPS[4]),
        ]
    )
    assert np.all(np.isin(v, [0.0, 1.0, 2.0]))
    return v.astype(np.float16)


VMATS16 = _build_vmats()


def _mk_ap(like_ap, offset, pattern):
    return RustAP(tensor=like_ap.tensor, offset=offset, ap=pattern)


def build_module(split_waits=True):
    nc = bass.Bass()
    # Input arrives host-padded: columns reflect-padded to 516.
    img = nc.dram_tensor("image", [NCH, H, XTW], F16, kind="ExternalInput")
    vm16 = nc.dram_tensor("vmats16", [3, 128, MW], F16, kind="ExternalInput")
    out = nc.dram_tensor("out", [NCH, H, W], F16, kind="ExternalOutput")

    with tile.TileContext(nc) as tc:
        with (
            tc.tile_pool(name="const", bufs=1) as constp,
            tc.tile_pool(name="xin", bufs=4) as xinp,
            tc.tile_pool(name="qp", bufs=4) as qpool,
            tc.tile_pool(name="ep", bufs=4) as epool,
            tc.tile_pool(name="outp", bufs=4) as outp,
            tc.tile_pool(name="psA", bufs=3, space=bass.MemorySpace.PSUM) as psA,
            tc.tile_pool(name="psT", bufs=2, space=bass.MemorySpace.PSUM) as psT,
        ):
            # Staging tile for the 14-row output tails of all planes: one
            # 3-level store at the end replaces 12 tiny per-plane stores.
            tstage = constp.tile([14, NCH * W], F16)
            tsv = tstage[:].rearrange("p (c f) -> p c f", c=NCH)

            # All planes' 16-row input tails in one upfront load (the
            # input is read-only): replaces 12 tiny per-plane descriptors.
            xtail = constp.tile([16, NCH * XTW], F16)
            xtv = xtail[:].rearrange("p (c f) -> p c f", c=NCH)
            nc.sync.dma_start(
                xtv,
                _mk_ap(
                    img[:],
                    (H - 16) * XTW,
                    [[XTW, 16], [H * XTW, NCH], [1, XTW]],
                ),
            )

            vt = constp.tile([128, 3 * MW], F16)
            vtr = vt[:].rearrange("p (i m) -> p i m", i=3)
            nc.sync.dma_start(
                vtr, _mk_ap(vm16[:], 0, [[MW, 128], [128 * MW, 3], [1, MW]])
            )

            # Warmup matmul consumes the weight tile right after its DMA.
            wup = psT.tile([128, 512], F32, tag="pt")
            warm = nc.tensor.matmul(
                wup[0:1, 0 : 3 * MW],
                vt[0:128, 0:1],
                vt[:],
                start=True,
                stop=True,
            )
            prev_mm = warm
            prev_dve = None
            prev_act = None
            prev_gps = None

            def chain(inst, which):
                nonlocal prev_dve, prev_act, prev_gps
                prevs = {"dve": prev_dve, "act": prev_act, "gps": prev_gps}
                p = prevs[which]
                if p is not None:
                    add_dep_helper(inst.ins, p.ins, sync=False, reason=which)
                if which == "dve":
                    prev_dve = inst
                elif which == "act":
                    prev_act = inst
                else:
                    prev_gps = inst
                return inst

            def mm_chain(inst):
                nonlocal prev_mm
                add_dep_helper(inst.ins, prev_mm.ins, sync=False, reason="pe order")
                prev_mm = inst
                return inst

            # DMA queues are FIFO per issuing engine: a store descriptor
            # waiting on an evac would block the NEXT plane's load sitting
            # behind it in the same queue. Issue loads PREFETCH planes
            # ahead of the compute so loads never queue behind stores.
            PREFETCH = 3
            xtiles = {}

            def issue_loads(c):
                x = xinp.tile([128, 4 * XTW], F16)
                xv = x[:].rearrange("p (a f) -> p a f", a=4)
                xtiles[c] = xv
                # Main load (chunks 0-3, 128 rows at stride 124) via
                # SWDGE: HWDGE HBM-read packets are confined to SDMA
                # engines 64-67, while SWDGE spreads across all 16.
                nc.gpsimd.dma_start(
                    xv[:, :, :],
                    _mk_ap(
                        img[:],
                        c * H * XTW,
                        [[XTW, 128], [124 * XTW, 4], [1, XTW]],
                    ),
                )

            for c in range(min(PREFETCH, NCH)):
                issue_loads(c)

            for c in range(NCH):
                if c + PREFETCH < NCH:
                    issue_loads(c + PREFETCH)
                xv = xtiles.pop(c)

                # Q[f] = Xp[f] + Xp[f+2]  (2x packed). Main chunks 0-3 at
                # full partition width; the 16-partition tail chunk runs
                # as its own small op (no memset of garbage partitions
                # needed that way).
                q = qpool.tile([128, 5 * QW], F16)
                qv = q[:].rearrange("p (a f) -> p a f", a=5)
                chain(
                    nc.vector.tensor_tensor(
                        qv[:, 0:4, 0:QW],
                        xv[:, 0:4, 0:QW],
                        xv[:, 0:4, 2 : 2 + QW],
                        mybir.AluOpType.add,
                    ),
                    "dve",
                )
                chain(
                    nc.vector.tensor_tensor(
                        qv[0:16, 4, 0:QW],
                        xtv[0:16, c, 0:QW],
                        xtv[0:16, c, 2 : 2 + QW],
                        mybir.AluOpType.add,
                    ),
                    "dve",
                )

                # E[w] = Q[w] + Xp[w+4]  (2x packed). The remaining Q[w+1]
                # term is a second accumulating matmul per group (the PE
                # streams the rhs at an odd element offset for free).
                e = epool.tile([128, 5 * W], F16)
                ev = e[:].rearrange("p (a f) -> p a f", a=5)
                chain(
                    nc.vector.tensor_tensor(
                        ev[:, 0:4, 0:W],
                        qv[:, 0:4, 0:W],
                        xv[:, 0:4, 4 : 4 + W],
                        mybir.AluOpType.add,
                    ),
                    "dve",
                )
                chain(
                    nc.vector.tensor_tensor(
                        ev[0:16, 4, 0:W],
                        qv[0:16, 4, 0:W],
                        xtv[0:16, c, 4 : 4 + W],
                        mybir.AluOpType.add,
                    ),
                    "dve",
                )

                # Vertical band matmuls, 2 accumulating per group:
                # PSUM_g = V_g^T @ E_g + V_g^T @ Q_g[w+1]
                pa1 = psA.tile([128, 1024], F32, tag="pa")
                pa2 = psA.tile([128, 1024], F32, tag="pa")
                pt = psT.tile([128, 512], F32, tag="pt")
                for g, (in_base, krows, out_base, m_rows) in enumerate(GROUPS):
                    # Write the full partition range the paired evac reads
                    # (extra weight columns are zero, so extra rows are 0).
                    mw = 126 if g < 4 else 14
                    kk = 128 if g < 4 else 16
                    if g < 4:
                        dst = (pa1 if g < 2 else pa2)[
                            0:mw, (g % 2) * W : (g % 2 + 1) * W
                        ]
                    else:
                        dst = pt[0:mw, :]
                    lhs = vtr[0:kk, _VM_IDX[g], 0:mw]
                    mm_chain(
                        nc.tensor.matmul(
                            dst,
                            lhs,
                            ev[0:kk, g, 0:W],
                            start=True,
                            stop=False,
                        )
                    )
                    mm_chain(
                        nc.tensor.matmul(
                            dst,
                            lhs,
                            qv[0:kk, g, 1 : 1 + W],
                            start=False,
                            stop=True,
                        )
                    )

                # Evacuate PSUM -> SBUF fp16 with the 1/25 scale on ScalarE.
                ot = outp.tile([128, 5 * W], F16)
                chain(nc.scalar.mul(ot[0:126, 0:1024], pa1[0:126, :], INV_AREA), "act")
                chain(
                    nc.scalar.mul(ot[0:126, 1024:2048], pa2[0:126, :], INV_AREA),
                    "act",
                )
                chain(
                    nc.scalar.mul(tsv[0:14, c, :], pt[0:14, :], INV_AREA), "act"
                )

                # Stores: g0 is 2-level on qSync HWDGE (engines 68-79);
                # the 3-level g1-g3 descriptor goes via SWDGE (multi-level
                # HWDGE descriptors pin to engines 64-67) except for the
                # last planes, where Sync is idle and SWDGE would extend
                # the end-of-kernel drain.
                ov = ot[:].rearrange("p (g f) -> p g f", g=5)
                nc.sync.dma_start(out[c, 0:126, :], ov[0:126, 0, :])
                midq = nc.gpsimd if c < NCH - 2 else nc.sync
                midq.dma_start(
                    _mk_ap(
                        out[:],
                        c * H * W + 126 * W,
                        [[W, 124], [124 * W, 3], [1, W]],
                    ),
                    ov[0:124, 1:4, :],
                )

            # Single 3-level store for all planes' 14-row tails.
            nc.sync.dma_start(
                _mk_ap(
                    out[:],
                    (H - 14) * W,
                    [[W, 14], [H * W, NCH], [1, W]],
                ),
                tsv[0:14, :, :],
            )

    if split_waits:
        _split_waits(nc)
    return nc


def _split_waits(nc):
    """Walrus legalization: each 64B ISA instruction has ONE sync-wait slot.

    Tile emits instructions with multiple semaphore waits; split the extras
    into standalone InstEventSemaphore sequencer waits (same engine queue,
    immediately before the instruction) which is semantically identical.
    """
    for fn in nc.m.functions:
        for b in fn.blocks:
            insts = b.instructions
            if not any(
                ins.sync_info and len(ins.sync_info.on_wait) > 1 for ins in insts
            ):
                continue
            new = []
            for ins in insts:
                si = ins.sync_info
                if si and len(si.on_wait) > 1:
                    waits = list(si.on_wait)
                    for w in waits[:-1]:
                        ev = mybir.InstEventSemaphore(
                            name=nc.get_next_instruction_name(),
                            engine=ins.engine,
                            ins=[],
                            outs=[],
                        )
                        ev.sync_info = mybir.SyncInfo(on_wait=[w], on_update=[])
                        new.append(ev)
                    si.on_wait = [waits[-1]]
                new.append(ins)
            b.instructions = new


_NC_CACHE = None


def _get_module():
    global _NC_CACHE
    if _NC_CACHE is None:
        _NC_CACHE = build_module()
    return _NC_CACHE


def kernel(image, _trace=False, _trace_kwargs=None):
    image = np.asarray(image)
    assert image.shape == (NB, 3, H, W), image.shape
    in_dtype = image.dtype
    img16 = np.pad(
        image.astype(np.float16),
        ((0, 0), (0, 0), (0, 0), (PAD, PAD)),
        mode="reflect",
    )

    nc = _get_module()
    in_maps = [
        {
            "image": img16[i * NBPC : (i + 1) * NBPC].reshape(NCH, H, XTW),
            "vmats16": VMATS16,
        }
        for i in range(N_CORES)
    ]
    res = run_bass_kernel_spmd(
        nc,
        in_maps,
        list(range(N_CORES)),
        trace=_trace,
        **(_trace_kwargs or {}),
    )
    full = np.concatenate(
        [res.results[i]["out"].reshape(NBPC, 3, H, W) for i in range(N_CORES)],
        axis=0,
    )
    out = full.astype(in_dtype, copy=False)
    if _trace:
        return out, res
    return out


# revision 35
# speedup vs baseline: 1.0797x; 1.0622x over previous
"""LocalMean 5x5 box filter (reflect pad) on TRN2, data-parallel over 8 cores.

Full input:  image (32, 3, 512, 512) fp32
Full output: same shape, 5x5 mean with reflect padding on H and W.

Sharding: batch dim 32 -> 4 images per core (12 channel planes of 512x512).

Design (the harness gate is rel_err < 2e-2, so no exact hi/lo split is
needed; measured rel_err ~9e-4):
  - Host converts input fp32->fp16 and reflect-pads the columns to 516;
    output is fp16 on device, upconverted on the host. Device HBM
    traffic is all fp16 (halves bytes, no on-device dtype conversion or
    padding).
  - Horizontal 5-tap: DVE computes Q[f]=Xp[f]+Xp[f+2] and E[w]=Q[w]+
    Xp[w+4], both in the 2x-packed 16-bit mode. The remaining Q[w+1]
    term is a second accumulating matmul per group (the PE streams the
    rhs at an odd element offset for free, same band weights).
  - Vertical 5-tap via PE band matmuls: groups of rows
    (126,124,124,124,14), K=(128x4,16), weights {0,1,2} exact in fp16.
  - PSUM->SBUF evacuation on ScalarE applies the 1/25 scale and the
    fp16 downconvert in one ACTIVATE per 2-bank PSUM pair.
  - DMA engine mapping (measured): HWDGE HBM-read packets are confined
    to SDMA engines 64-67 and multi-level HWDGE descriptors also pin to
    64-67, while SWDGE (gpsimd dma_start) spreads across all 16 engines
    and 2-level HWDGE writes spread over 68-79. Hence: main loads and
    the 3-level middle store go via SWDGE; g0 stores via qSync; all
    input tails are fetched in one upfront descriptor and all output
    tails staged in SBUF and stored in one final descriptor.
  - DMA queues are FIFO per issuing engine, so loads are issued 3
    planes ahead of compute to never sit behind a store descriptor
    that is still waiting on its evacuation.
"""

import numpy as np

import concourse.bass as bass
import concourse.mybir as mybir
import concourse.tile as tile
from concourse.tile import add_dep_helper
from concourse.bass_utils import run_bass_kernel_spmd

try:
    from bass_rust import AP as RustAP
except ImportError:  # pragma: no cover
    RustAP = None

F32 = mybir.dt.float32
F16 = mybir.dt.float16

N_CORES = 8
NB = 32
NBPC = NB // N_CORES
NCH = NBPC * 3
H = W = 512
PATCH = 5
PAD = 2
INV_AREA = 1.0 / float(PATCH * PATCH)

XTW = W + 2 * PAD  # 516 padded width
QW = W + 2  # 514 gap-pair width

# Row groups: (in_base, K, out_base, M). Chunks 0-3 are 128 rows at
# stride 124; the tail chunk is rows 496..511 (K=16).
GROUPS = [
    (0, 128, 0, 126),
    (124, 128, 126, 124),
    (248, 128, 250, 124),
    (372, 128, 374, 124),
    (496, 16, 498, 14),
]
_VM_IDX = [0, 1, 1, 1, 2]
MW = 126  # max group M (weight tile column count)


def _reflect(t, n):
    if t < 0:
        t = -t
    if t > n - 1:
        t = 2 * (n - 1) - t
    return t


def _v_matrix(in_base, k_rows, out_base, m_rows):
    v = np.zeros((128, MW), np.float32)
    for m in range(m_rows):
        r = out_base + m
        for t in range(r - PAD, r + PAD + 1):
            k = _reflect(t, H) - in_base
            assert 0 <= k < k_rows, (r, t, k)
            v[k, m] += 1.0
    return v


def _build_vmats():
    v = np.stack(
        [
            _v_matrix(*GROUPS[0]),
            _v_matrix(*GROUPS[1]),
            _v_matrix(*GROUPS[4]),
        ]
    )
    assert np.all(np.isin(v, [0.0, 1.0, 2.0]))
    return v.astype(np.float16)


VMATS16 = _build_vmats()


def _mk_ap(like_ap, offset, pattern):
    return RustAP(tensor=like_ap.tensor, offset=offset, ap=pattern)


def build_module(split_waits=True):
    nc = bass.Bass()
    # Input arrives host-padded: columns reflect-padded to 516.
    img = nc.dram_tensor("image", [NCH, H, XTW], F16, kind="ExternalInput")
    vm16 = nc.dram_tensor("vmats16", [3, 128, MW], F16, kind="ExternalInput")
    out = nc.dram_tensor("out", [NCH, H, W], F16, kind="ExternalOutput")

    with tile.TileContext(nc) as tc:
        with (
            tc.tile_pool(name="const", bufs=1) as constp,
            tc.tile_pool(name="xin", bufs=6) as xinp,
            tc.tile_pool(name="qp", bufs=4) as qpool,
            tc.tile_pool(name="ep", bufs=4) as epool,
            tc.tile_pool(name="outp", bufs=6) as outp,
            tc.tile_pool(name="psA", bufs=3, space=bass.MemorySpace.PSUM) as psA,
            tc.tile_pool(name="psT", bufs=2, space=bass.MemorySpace.PSUM) as psT,
        ):
            # Staging tile for the 14-row output tails of all planes: one
            # 3-level store at the end replaces 12 tiny per-plane stores.
            tstage = constp.tile([14, NCH * W], F16)
            tsv = tstage[:].rearrange("p (c f) -> p c f", c=NCH)

            # All planes' 16-row input tails in one upfront load (the
            # input is read-only): replaces 12 tiny per-plane descriptors.
            xtail = constp.tile([16, NCH * XTW], F16)
            xtv = xtail[:].rearrange("p (c f) -> p c f", c=NCH)
            nc.sync.dma_start(
                xtv,
                _mk_ap(
                    img[:],
                    (H - 16) * XTW,
                    [[XTW, 16], [H * XTW, NCH], [1, XTW]],
                ),
            )

            vt = constp.tile([128, 3 * MW], F16)
            vtr = vt[:].rearrange("p (i m) -> p i m", i=3)
            nc.sync.dma_start(
                vtr, _mk_ap(vm16[:], 0, [[MW, 128], [128 * MW, 3], [1, MW]])
            )

            # Warmup matmul consumes the weight tile right after its DMA.
            wup = psT.tile([128, 512], F32, tag="pt")
            warm = nc.tensor.matmul(
                wup[0:1, 0 : 3 * MW],
                vt[0:128, 0:1],
                vt[:],
                start=True,
                stop=True,
            )
            prev_mm = warm
            prev_dve = None
            prev_act = None
            prev_gps = None

            def chain(inst, which):
                nonlocal prev_dve, prev_act, prev_gps
                prevs = {"dve": prev_dve, "act": prev_act, "gps": prev_gps}
                p = prevs[which]
                if p is not None:
                    add_dep_helper(inst.ins, p.ins, sync=False, reason=which)
                if which == "dve":
                    prev_dve = inst
                elif which == "act":
                    prev_act = inst
                else:
                    prev_gps = inst
                return inst

            def mm_chain(inst):
                nonlocal prev_mm
                add_dep_helper(inst.ins, prev_mm.ins, sync=False, reason="pe order")
                prev_mm = inst
                return inst

            # DMA queues are FIFO per issuing engine: a store descriptor
            # waiting on an evac would block the NEXT plane's load sitting
            # behind it in the same queue. Issue loads PREFETCH planes
            # ahead of the compute so loads never queue behind stores.
            PREFETCH = 3
            xtiles = {}

            def issue_loads(c):
                x = xinp.tile([128, 4 * XTW], F16)
                xv = x[:].rearrange("p (a f) -> p a f", a=4)
                xtiles[c] = xv
                # Main load (chunks 0-3, 128 rows at stride 124) via
                # SWDGE: HWDGE HBM-read packets are confined to SDMA
                # engines 64-67, while SWDGE spreads across all 16.
                nc.gpsimd.dma_start(
                    xv[:, :, :],
                    _mk_ap(
                        img[:],
                        c * H * XTW,
                        [[XTW, 128], [124 * XTW, 4], [1, XTW]],
                    ),
                )

            for c in range(min(PREFETCH, NCH)):
                issue_loads(c)

            for c in range(NCH):
                if c + PREFETCH < NCH:
                    issue_loads(c + PREFETCH)
                xv = xtiles.pop(c)

                # Q[f] = Xp[f] + Xp[f+2]  (2x packed). Main chunks 0-3 at
                # full partition width; the 16-partition tail chunk runs
                # as its own small op (no memset of garbage partitions
                # needed that way).
                q = qpool.tile([128, 5 * QW], F16)
                qv = q[:].rearrange("p (a f) -> p a f", a=5)
                chain(
                    nc.vector.tensor_tensor(
                        qv[:, 0:4, 0:QW],
                        xv[:, 0:4, 0:QW],
                        xv[:, 0:4, 2 : 2 + QW],
                        mybir.AluOpType.add,
                    ),
                    "dve",
                )
                chain(
                    nc.vector.tensor_tensor(
                        qv[0:16, 4, 0:QW],
                        xtv[0:16, c, 0:QW],
                        xtv[0:16, c, 2 : 2 + QW],
                        mybir.AluOpType.add,
                    ),
                    "dve",
                )

                # E[w] = Q[w] + Xp[w+4]  (2x packed). The remaining Q[w+1]
                # term is a second accumulating matmul per group (the PE
                # streams the rhs at an odd element offset for free).
                e = epool.tile([128, 5 * W], F16)
                ev = e[:].rearrange("p (a f) -> p a f", a=5)
                chain(
                    nc.vector.tensor_tensor(
                        ev[:, 0:4, 0:W],
                        qv[:, 0:4, 0:W],
                        xv[:, 0:4, 4 : 4 + W],
                        mybir.AluOpType.add,
                    ),
                    "dve",
                )
                chain(
                    nc.vector.tensor_tensor(
                        ev[0:16, 4, 0:W],
                        qv[0:16, 4, 0:W],
                        xtv[0:16, c, 4 : 4 + W],
                        mybir.AluOpType.add,
                    ),
                    "dve",
                )

                # Vertical band matmuls, 2 accumulating per group:
                # PSUM_g = V_g^T @ E_g + V_g^T @ Q_g[w+1]
                pa1 = psA.tile([128, 1024], F32, tag="pa")
                pa2 = psA.tile([128, 1024], F32, tag="pa")
                pt = psT.tile([128, 512], F32, tag="pt")
                for g, (in_base, krows, out_base, m_rows) in enumerate(GROUPS):
                    # Write the full partition range the paired evac reads
                    # (extra weight columns are zero, so extra rows are 0).
                    mw = 126 if g < 4 else 14
                    kk = 128 if g < 4 else 16
                    if g < 4:
                        dst = (pa1 if g < 2 else pa2)[
                            0:mw, (g % 2) * W : (g % 2 + 1) * W
                        ]
                    else:
                        dst = pt[0:mw, :]
                    lhs = vtr[0:kk, _VM_IDX[g], 0:mw]
                    mm_chain(
                        nc.tensor.matmul(
                            dst,
                            lhs,
                            ev[0:kk, g, 0:W],
                            start=True,
                            stop=False,
                        )
                    )
                    mm_chain(
                        nc.tensor.matmul(
                            dst,
                            lhs,
                            qv[0:kk, g, 1 : 1 + W],
                            start=False,
                            stop=True,
                        )
                    )

                # Evacuate PSUM -> SBUF fp16 with the 1/25 scale on ScalarE.
                ot = outp.tile([128, 5 * W], F16)
                chain(nc.scalar.mul(ot[0:126, 0:1024], pa1[0:126, :], INV_AREA), "act")
                chain(
                    nc.scalar.mul(ot[0:126, 1024:2048], pa2[0:126, :], INV_AREA),
                    "act",
                )
                chain(
                    nc.scalar.mul(tsv[0:14, c, :], pt[0:14, :], INV_AREA), "act"
                )

                # Stores: g0 is 2-level on qSync HWDGE (engines 68-79);
                # the 3-level g1-g3 descriptor goes via SWDGE (multi-level
                # HWDGE descriptors pin to engines 64-67).
                ov = ot[:].rearrange("p (g f) -> p g f", g=5)
                nc.sync.dma_start(out[c, 0:126, :], ov[0:126, 0, :])
                nc.gpsimd.dma_start(
                    _mk_ap(
                        out[:],
                        c * H * W + 126 * W,
                        [[W, 124], [124 * W, 3], [1, W]],
                    ),
                    ov[0:124, 1:4, :],
                )

            # Single 3-level store for all planes' 14-row tails.
            nc.sync.dma_start(
                _mk_ap(
                    out[:],
                    (H - 14) * W,
                    [[W, 14], [H * W, NCH], [1, W]],
                ),
                tsv[0:14, :, :],
            )

    if split_waits:
        _split_waits(nc)
    return nc


def _split_waits(nc):
    """Walrus legalization: each 64B ISA instruction has ONE sync-wait slot.

    Tile emits instructions with multiple semaphore waits; split the extras
    into standalone InstEventSemaphore sequencer waits (same engine queue,
    immediately before the instruction) which is semantically identical.
    """
    for fn in nc.m.functions:
        for b in fn.blocks:
            insts = b.instructions
            if not any(
                ins.sync_info and len(ins.sync_info.on_wait) > 1 for ins in insts
            ):
                continue
            new = []
            for ins in insts:
                si = ins.sync_info
                if si and len(si.on_wait) > 1:
                    waits = list(si.on_wait)
                    for w in waits[:-1]:
                        ev = mybir.InstEventSemaphore(
                            name=nc.get_next_instruction_name(),
                            engine=ins.engine,
                            ins=[],
                            outs=[],
                        )
                        ev.sync_info = mybir.SyncInfo(on_wait=[w], on_update=[])
                        new.append(ev)
                    si.on_wait = [waits[-1]]
                new.append(ins)
            b.instructions = new


_NC_CACHE = None


def _get_module():
    global _NC_CACHE
    if _NC_CACHE is None:
        _NC_CACHE = build_module()
    return _NC_CACHE


def kernel(image, _trace=False, _trace_kwargs=None):
    image = np.asarray(image)
    assert image.shape == (NB, 3, H, W), image.shape
    in_dtype = image.dtype
    img16 = np.pad(
        image.astype(np.float16),
        ((0, 0), (0, 0), (0, 0), (PAD, PAD)),
        mode="reflect",
    )

    nc = _get_module()
    in_maps = [
        {
            "image": img16[i * NBPC : (i + 1) * NBPC].reshape(NCH, H, XTW),
            "vmats16": VMATS16,
        }
        for i in range(N_CORES)
    ]
    res = run_bass_kernel_spmd(
        nc,
        in_maps,
        list(range(N_CORES)),
        trace=_trace,
        **(_trace_kwargs or {}),
    )
    full = np.concatenate(
        [res.results[i]["out"].reshape(NBPC, 3, H, W) for i in range(N_CORES)],
        axis=0,
    )
    out = full.astype(in_dtype, copy=False)
    if _trace:
        return out, res
    return out


# revision 37
# speedup vs baseline: 1.1056x; 1.0240x over previous
"""LocalMean 5x5 box filter (reflect pad) on TRN2, data-parallel over 8 cores.

Full input:  image (32, 3, 512, 512) fp32
Full output: same shape, 5x5 mean with reflect padding on H and W.

Sharding: batch dim 32 -> 4 images per core (12 channel planes of 512x512).

Design (the harness gate is rel_err < 2e-2, so no exact hi/lo split is
needed; measured rel_err ~9e-4):
  - Host converts input fp32->fp16 and reflect-pads the columns to 516;
    output is fp16 on device, upconverted on the host. Device HBM
    traffic is all fp16 (halves bytes, no on-device dtype conversion or
    padding).
  - Horizontal 5-tap: DVE computes Q[f]=Xp[f]+Xp[f+2] and E[w]=Q[w]+
    Xp[w+4], both in the 2x-packed 16-bit mode. The remaining Q[w+1]
    term is a second accumulating matmul per group (the PE streams the
    rhs at an odd element offset for free, same band weights).
  - Vertical 5-tap via PE band matmuls: groups of rows
    (126,124,124,124,14), K=(128x4,16), weights {0,1,2} exact in fp16.
  - PSUM->SBUF evacuation on ScalarE applies the 1/25 scale and the
    fp16 downconvert in one ACTIVATE per 2-bank PSUM pair.
  - DMA engine mapping (measured): HWDGE HBM-read packets are confined
    to SDMA engines 64-67 and multi-level HWDGE descriptors also pin to
    64-67, while SWDGE (gpsimd dma_start) spreads across all 16 engines
    and 2-level HWDGE writes spread over 68-79. Hence: main loads and
    the 3-level middle store go via SWDGE; g0 stores via qSync; all
    input tails are fetched in one upfront descriptor and all output
    tails staged in SBUF and stored in one final descriptor.
  - DMA queues are FIFO per issuing engine, so loads are issued 3
    planes ahead of compute to never sit behind a store descriptor
    that is still waiting on its evacuation.
"""

import numpy as np

import concourse.bass as bass
import concourse.mybir as mybir
import concourse.tile as tile
from concourse.tile import add_dep_helper
from concourse.bass_utils import run_bass_kernel_spmd

try:
    from bass_rust import AP as RustAP
except ImportError:  # pragma: no cover
    RustAP = None

F32 = mybir.dt.float32
F16 = mybir.dt.float16

N_CORES = 8
NB = 32
NBPC = NB // N_CORES
NCH = NBPC * 3
H = W = 512
PATCH = 5
PAD = 2
INV_AREA = 1.0 / float(PATCH * PATCH)

XTW = W + 2 * PAD  # 516 padded width
QW = W + 2  # 514 gap-pair width

# Row groups: (in_base, K, out_base, M). Chunks 0-3 are 128 rows at
# stride 124; the tail chunk is rows 496..511 (K=16).
GROUPS = [
    (0, 128, 0, 126),
    (124, 128, 126, 124),
    (248, 128, 250, 124),
    (372, 128, 374, 124),
    (496, 16, 498, 14),
]
_VM_IDX = [0, 1, 1, 1, 2]
MW = 126  # max group M (weight tile column count)


def _reflect(t, n):
    if t < 0:
        t = -t
    if t > n - 1:
        t = 2 * (n - 1) - t
    return t


def _v_matrix(in_base, k_rows, out_base, m_rows):
    v = np.zeros((128, MW), np.float32)
    for m in range(m_rows):
        r = out_base + m
        for t in range(r - PAD, r + PAD + 1):
            k = _reflect(t, H) - in_base
            assert 0 <= k < k_rows, (r, t, k)
            v[k, m] += 1.0
    return v


def _build_vmats():
    v = np.stack(
        [
            _v_matrix(*GROUPS[0]),
            _v_matrix(*GROUPS[1]),
            _v_matrix(*GROUPS[4]),
        ]
    )
    assert np.all(np.isin(v, [0.0, 1.0, 2.0]))
    return v.astype(np.float16)


VMATS16 = _build_vmats()


def _mk_ap(like_ap, offset, pattern):
    return RustAP(tensor=like_ap.tensor, offset=offset, ap=pattern)


def build_module(split_waits=True):
    nc = bass.Bass()
    # Input arrives host-padded: columns reflect-padded to 516.
    img = nc.dram_tensor("image", [NCH, H, XTW], F16, kind="ExternalInput")
    vm16 = nc.dram_tensor("vmats16", [3, 128, MW], F16, kind="ExternalInput")
    out = nc.dram_tensor("out", [NCH, H, W], F16, kind="ExternalOutput")

    with tile.TileContext(nc) as tc:
        with (
            tc.tile_pool(name="const", bufs=1) as constp,
            tc.tile_pool(name="xin", bufs=6) as xinp,
            tc.tile_pool(name="qp", bufs=4) as qpool,
            tc.tile_pool(name="ep", bufs=4) as epool,
            tc.tile_pool(name="outp", bufs=6) as outp,
            tc.tile_pool(name="psA", bufs=3, space=bass.MemorySpace.PSUM) as psA,
            tc.tile_pool(name="psT", bufs=2, space=bass.MemorySpace.PSUM) as psT,
        ):
            # Staging tile for the 14-row output tails of all planes: one
            # 3-level store at the end replaces 12 tiny per-plane stores.
            tstage = constp.tile([14, NCH * W], F16)
            tsv = tstage[:].rearrange("p (c f) -> p c f", c=NCH)

            # Tiny SWDGE warmup DMA: the first SWDGE use pays Q7/ring
            # init cost; spend it on a throwaway transfer so the first
            # real plane load starts promptly.
            wdma = constp.tile([16, XTW], F16)
            nc.gpsimd.dma_start(wdma[0:16, :], img[0, 0:16, :])

            # All planes' 16-row input tails in one upfront load (the
            # input is read-only): replaces 12 tiny per-plane descriptors.
            xtail = constp.tile([16, NCH * XTW], F16)
            xtv = xtail[:].rearrange("p (c f) -> p c f", c=NCH)
            nc.sync.dma_start(
                xtv,
                _mk_ap(
                    img[:],
                    (H - 16) * XTW,
                    [[XTW, 16], [H * XTW, NCH], [1, XTW]],
                ),
            )

            vt = constp.tile([128, 3 * MW], F16)
            vtr = vt[:].rearrange("p (i m) -> p i m", i=3)
            nc.sync.dma_start(
                vtr, _mk_ap(vm16[:], 0, [[MW, 128], [128 * MW, 3], [1, MW]])
            )

            # Warmup matmul consumes the weight tile right after its DMA.
            wup = psT.tile([128, 512], F32, tag="pt")
            warm = nc.tensor.matmul(
                wup[0:1, 0 : 3 * MW],
                vt[0:128, 0:1],
                vt[:],
                start=True,
                stop=True,
            )
            prev_mm = warm
            prev_dve = None
            prev_act = None
            prev_gps = None

            def chain(inst, which):
                nonlocal prev_dve, prev_act, prev_gps
                prevs = {"dve": prev_dve, "act": prev_act, "gps": prev_gps}
                p = prevs[which]
                if p is not None:
                    add_dep_helper(inst.ins, p.ins, sync=False, reason=which)
                if which == "dve":
                    prev_dve = inst
                elif which == "act":
                    prev_act = inst
                else:
                    prev_gps = inst
                return inst

            def mm_chain(inst):
                nonlocal prev_mm
                add_dep_helper(inst.ins, prev_mm.ins, sync=False, reason="pe order")
                prev_mm = inst
                return inst

            # DMA queues are FIFO per issuing engine: a store descriptor
            # waiting on an evac would block the NEXT plane's load sitting
            # behind it in the same queue. Issue loads PREFETCH planes
            # ahead of the compute so loads never queue behind stores.
            PREFETCH = 5
            xtiles = {}

            def issue_loads(c):
                x = xinp.tile([128, 4 * XTW], F16)
                xv = x[:].rearrange("p (a f) -> p a f", a=4)
                xtiles[c] = xv
                # Main load (chunks 0-3, 128 rows at stride 124) via
                # SWDGE: HWDGE HBM-read packets are confined to SDMA
                # engines 64-67, while SWDGE spreads across all 16.
                nc.gpsimd.dma_start(
                    xv[:, :, :],
                    _mk_ap(
                        img[:],
                        c * H * XTW,
                        [[XTW, 128], [124 * XTW, 4], [1, XTW]],
                    ),
                )

            for c in range(min(PREFETCH, NCH)):
                issue_loads(c)

            for c in range(NCH):
                if c + PREFETCH < NCH:
                    issue_loads(c + PREFETCH)
                xv = xtiles.pop(c)

                # Q[f] = Xp[f] + Xp[f+2]  (2x packed). Main chunks 0-3 at
                # full partition width; the 16-partition tail chunk runs
                # as its own small op (no memset of garbage partitions
                # needed that way).
                q = qpool.tile([128, 5 * QW], F16)
                qv = q[:].rearrange("p (a f) -> p a f", a=5)
                chain(
                    nc.vector.tensor_tensor(
                        qv[:, 0:4, 0:QW],
                        xv[:, 0:4, 0:QW],
                        xv[:, 0:4, 2 : 2 + QW],
                        mybir.AluOpType.add,
                    ),
                    "dve",
                )
                chain(
                    nc.vector.tensor_tensor(
                        qv[0:16, 4, 0:QW],
                        xtv[0:16, c, 0:QW],
                        xtv[0:16, c, 2 : 2 + QW],
                        mybir.AluOpType.add,
                    ),
                    "dve",
                )

                # E[w] = Q[w] + Xp[w+4]  (2x packed). The remaining Q[w+1]
                # term is a second accumulating matmul per group (the PE
                # streams the rhs at an odd element offset for free).
                e = epool.tile([128, 5 * W], F16)
                ev = e[:].rearrange("p (a f) -> p a f", a=5)
                chain(
                    nc.vector.tensor_tensor(
                        ev[:, 0:4, 0:W],
                        qv[:, 0:4, 0:W],
                        xv[:, 0:4, 4 : 4 + W],
                        mybir.AluOpType.add,
                    ),
                    "dve",
                )
                chain(
                    nc.vector.tensor_tensor(
                        ev[0:16, 4, 0:W],
                        qv[0:16, 4, 0:W],
                        xtv[0:16, c, 4 : 4 + W],
                        mybir.AluOpType.add,
                    ),
                    "dve",
                )

                # Vertical band matmuls, 2 accumulating per group:
                # PSUM_g = V_g^T @ E_g + V_g^T @ Q_g[w+1]
                pa1 = psA.tile([128, 1024], F32, tag="pa")
                pa2 = psA.tile([128, 1024], F32, tag="pa")
                pt = psT.tile([128, 512], F32, tag="pt")
                for g, (in_base, krows, out_base, m_rows) in enumerate(GROUPS):
                    # Write the full partition range the paired evac reads
                    # (extra weight columns are zero, so extra rows are 0).
                    mw = 126 if g < 4 else 14
                    kk = 128 if g < 4 else 16
                    if g < 4:
                        dst = (pa1 if g < 2 else pa2)[
                            0:mw, (g % 2) * W : (g % 2 + 1) * W
                        ]
                    else:
                        dst = pt[0:mw, :]
                    lhs = vtr[0:kk, _VM_IDX[g], 0:mw]
                    mm_chain(
                        nc.tensor.matmul(
                            dst,
                            lhs,
                            ev[0:kk, g, 0:W],
                            start=True,
                            stop=False,
                        )
                    )
                    mm_chain(
                        nc.tensor.matmul(
                            dst,
                            lhs,
                            qv[0:kk, g, 1 : 1 + W],
                            start=False,
                            stop=True,
                        )
                    )

                # Evacuate PSUM -> SBUF fp16 with the 1/25 scale on ScalarE.
                ot = outp.tile([128, 5 * W], F16)
                chain(nc.scalar.mul(ot[0:126, 0:1024], pa1[0:126, :], INV_AREA), "act")
                chain(
                    nc.scalar.mul(ot[0:126, 1024:2048], pa2[0:126, :], INV_AREA),
                    "act",
                )
                chain(
                    nc.scalar.mul(tsv[0:14, c, :], pt[0:14, :], INV_AREA), "act"
                )

                # Stores: g0 is 2-level on qSync HWDGE (engines 68-79);
                # the 3-level g1-g3 descriptor goes via SWDGE (multi-level
                # HWDGE descriptors pin to engines 64-67).
                ov = ot[:].rearrange("p (g f) -> p g f", g=5)
                nc.sync.dma_start(out[c, 0:126, :], ov[0:126, 0, :])
                nc.gpsimd.dma_start(
                    _mk_ap(
                        out[:],
                        c * H * W + 126 * W,
                        [[W, 124], [124 * W, 3], [1, W]],
                    ),
                    ov[0:124, 1:4, :],
                )

            # Single 3-level store for all planes' 14-row tails.
            nc.sync.dma_start(
                _mk_ap(
                    out[:],
                    (H - 14) * W,
                    [[W, 14], [H * W, NCH], [1, W]],
                ),
                tsv[0:14, :, :],
            )

    if split_waits:
        _split_waits(nc)
    return nc


def _split_waits(nc):
    """Walrus legalization: each 64B ISA instruction has ONE sync-wait slot.

    Tile emits instructions with multiple semaphore waits; split the extras
    into standalone InstEventSemaphore sequencer waits (same engine queue,
    immediately before the instruction) which is semantically identical.
    """
    for fn in nc.m.functions:
        for b in fn.blocks:
            insts = b.instructions
            if not any(
                ins.sync_info and len(ins.sync_info.on_wait) > 1 for ins in insts
            ):
                continue
            new = []
            for ins in insts:
                si = ins.sync_info
                if si and len(si.on_wait) > 1:
                    waits = list(si.on_wait)
                    for w in waits[:-1]:
                        ev = mybir.InstEventSemaphore(
                            name=nc.get_next_instruction_name(),
                            engine=ins.engine,
                            ins=[],
                            outs=[],
                        )
                        ev.sync_info = mybir.SyncInfo(on_wait=[w], on_update=[])
                        new.append(ev)
                    si.on_wait = [waits[-1]]
                new.append(ins)
            b.instructions = new


_NC_CACHE = None


def _get_module():
    global _NC_CACHE
    if _NC_CACHE is None:
        _NC_CACHE = build_module()
    return _NC_CACHE


def kernel(image, _trace=False, _trace_kwargs=None):
    image = np.asarray(image)
    assert image.shape == (NB, 3, H, W), image.shape
    in_dtype = image.dtype
    img16 = np.pad(
        image.astype(np.float16),
        ((0, 0), (0, 0), (0, 0), (PAD, PAD)),
        mode="reflect",
    )

    nc = _get_module()
    in_maps = [
        {
            "image": img16[i * NBPC : (i + 1) * NBPC].reshape(NCH, H, XTW),
            "vmats16": VMATS16,
        }
        for i in range(N_CORES)
    ]
    res = run_bass_kernel_spmd(
        nc,
        in_maps,
        list(range(N_CORES)),
        trace=_trace,
        **(_trace_kwargs or {}),
    )
    full = np.concatenate(
        [res.results[i]["out"].reshape(NBPC, 3, H, W) for i in range(N_CORES)],
        axis=0,
    )
    out = full.astype(in_dtype, copy=False)
    if _trace:
        return out, res
    return out
